# revision 55
# baseline (speedup 1.0000x reference)
"""Trainium2 Bass kernel for quantized BasicBlock (DoReFa conv-bn-quant x2 + skip).

Strategy:
- Data-parallel over batch: 128 images -> 16 per core across 8 cores.
- Weights quantize on the HOST to odd integers in [-15,15] (numpy matches the
  jax reference bit-for-bit); the 1/15 (conv1) and 1/225 (conv2) scales fold
  into host-precomputed BN affines. Integer taps upload as bf16 (conv1, exact)
  and as raw fp8e4 bytes (conv2).
- conv1: f32r matmuls (fp22 multiply, 1 cycle/row at N=512) over a
  zero-padded f32r image (ACT/DVE pad-copy performs the required fp32r
  pre-rounding; the BIR verifier rejects raw-DMA-fed f32r operands).
- conv2: activations are 4-bit ints 0..15 and weights odd ints in [-15,15],
  both exact in fp8e4 -> conv2 is exact integer arithmetic. The 7 taps run
  as 4 fp8 DoubleRow matmuls (two K-tiles each: taps (ky,kx),(ky+1,kx) via a
  [C,2,16,32] overlapping-window AP; the 4th pairs tap (2,1) with a
  zero-weight dummy), i.e. 2 matmul-units instead of 7 per half-image.
- 3x3 conv with 2 pruned taps = 7 shifted matmuls [K=128,M=128,N=512]
  accumulated in PSUM over zero-padded [C,34,34] SBUF images.
- stage1 rounding via the +2^23 magic-add trick = IEEE RNE (jnp.round);
  stage2 clip+round rides the DVE fp32->uint8 conversion, which is RNE with
  low-saturation at 0 (HW-verified), after a single min-15.
- Output leaves the device as uint8 ints 0..15 (4x less DMA); the exact /15
  dequant is a 16-entry LUT on the host.
- Engine budget per image: PE 14 f32r + 8 DoubleRow matmuls; ACT pad-copy
  (even images) + affines r/g; DVE pad-copy (odd), round/clip chain, skip-add.
  gpsimd/Pool does NO elementwise compute: its TensorScalar ucode measures
  ~6.7us per [C,512] op on real HW (~8x the cost model), so only memsets go
  there. Software-pipelined with a one-image skew (conv1(i+1) ahead of
  conv2(i) in the in-order PE queue), NB=4 padded buffers, batched pair DMAs,
  and a PE p-state warmup stream (the cost model halves the clock until the
  PE has been busy ~3us).
"""
import numpy as np

import concourse.bass as bass
import concourse.tile as tile
import ml_dtypes
from concourse import bacc, mybir
from concourse.bass_utils import run_bass_kernel_spmd

AF = mybir.ActivationFunctionType
OP = mybir.AluOpType
F32 = mybir.dt.float32
F32R = mybir.dt.float32r
BF16 = mybir.dt.bfloat16
FP8 = mybir.dt.float8e4
DR = mybir.MatmulPerfMode.DoubleRow

B, C, H, W = 128, 128, 32, 32
NCORES = 8
BL = B // NCORES          # images per core
HP, WP = H + 2, W + 2     # zero-padded image
NPIX = H * W
TAPS = [(0, 1), (0, 2), (1, 0), (1, 1), (1, 2), (2, 0), (2, 1)]  # (0,0),(2,2) pruned
# conv2 tap pairing for fp8 DoubleRow: pairs differ by +1 row (offset delta WP)
PAIR2 = [((0, 1), (1, 1)), ((0, 2), (1, 2)), ((1, 0), (2, 0)),
         ((2, 1), (3, 1))]   # (3,1) is a zero-weight dummy k-tile
NPAIR = len(PAIR2)
MAGIC = float(2 ** 23)
EPS = 1e-5
NB = 4                    # padded-buffer pipeline depth (images)
SPOOL_BUFS = 6
IPOOL_BUFS = 3            # x staging buffers (2 images each)
OPOOL_BUFS = 3            # y staging buffers (2 images each)
U8 = mybir.dt.uint8
WARMUP = 5                # PE p-state warmup matmuls
SKIP_TAIL_N = 3           # images at the pipeline end using the PE skip
W1SPLIT = 2               # taps in the first (early) w1r pre-round copy
PS1_BUFS = 3              # conv1 PSUM half-tile buffers
PS2_BUFS = 3              # conv2 PSUM half-tile buffers
# --- HW bisect flags (default = fast path) ---
USE_DR = True             # conv2 DoubleRow pairs (False: 7 plain fp8 taps)
PAIR_DMA = True           # batched 2-image in-DMAs (False: per-image)
SKIP_PE = "tail"          # skip-connection engine. False = DVE
                          # scalar_tensor_tensor everywhere (2x SBUF mode);
                          # "tail" = PE diag matmul for the last two images
                          # only (their extra matmuls run in otherwise-idle
                          # PE time and drop hh from the latency-critical
                          # pipeline tail); True/"hybrid" = more PE variants

BN_NAMES = ["gamma1", "beta1", "mean1", "var1", "gamma2", "beta2", "mean2", "var2"]


def _pair_ap(padded, ky, kx, h):
    """Moving operand [C, 2(k-tile: taps (ky,kx),(ky+1,kx)), 16, 32] for DoubleRow."""
    base = padded[:]
    return bass.AP(base.tensor, base.offset + (16 * h + ky) * WP + kx,
                   [[base.ap[0][0], C], [WP, 2], [WP, 16], [1, W]])


def _emit(tc, dr, bl, repeat=1):
    nc = tc.nc
    with tc.tile_pool(name="const", bufs=1) as cpool, \
         tc.tile_pool(name="img", bufs=IPOOL_BUFS) as ipool, \
         tc.tile_pool(name="out", bufs=OPOOL_BUFS) as opool, \
         tc.tile_pool(name="stage", bufs=SPOOL_BUFS) as spool, \
         tc.tile_pool(name="ps1", bufs=PS1_BUFS, space="PSUM") as pp1, \
         tc.tile_pool(name="ps2", bufs=PS2_BUFS, space="PSUM") as pp2:

        # critical startup path: the DMA pipe is serial, so order transfers
        # by need: conv1 weights (small, bf16) first, then image 0 in row
        # blocks matching the split pad-copy, then image 1
        xsb2_0 = ipool.tile([C, 2, H, W], F32, tag="xsb2", name="xsb2_0")
        nc.sync.dma_start(xsb2_0[:, 0, 0:18], dr["x"][0][:, 0:18])
        w1sb = cpool.tile([C, 7, C], BF16, tag="w1sb")
        nc.sync.dma_start(w1sb[:], dr["w1t"])
        # conv1: bf16 staging -> split DVE copies perform the fp32r pre-round;
        # the first two taps arrive early so conv1(0) can launch sooner
        w1r = cpool.tile([C, 7, C], F32R, tag="w1r")
        nc.vector.tensor_copy(w1r[:, 0:W1SPLIT, :], w1sb[:, 0:W1SPLIT, :])
        nc.vector.tensor_copy(w1r[:, W1SPLIT:7, :], w1sb[:, W1SPLIT:7, :])
        w1T = [w1r[:, t, :] for t in range(7)]
        nc.sync.dma_start(xsb2_0[:, 0, 18:H], dr["x"][0][:, 18:H])
        nc.sync.dma_start(xsb2_0[:, 1], dr["x"][1])
        # conv2: fp8 bytes land directly; bitcast views for the matmuls
        w2sb = cpool.tile([C, 8, C], U8, tag="w2sb")
        nc.sync.dma_start(w2sb[:], dr["w2q"])
        wp2 = [w2sb[:, 2 * p:2 * p + 2, :].bitcast(FP8) for p in range(NPAIR)]

        # BN affines precomputed on host: [inv1, bs1, sc2, bs2]
        bna = cpool.tile([C, 4], F32, tag="bna")
        nc.sync.dma_start(bna[:], dr["bna"])
        inv1 = bna[:, 0:1]
        sc2 = bna[:, 2:3]
        b_s = {"1": bna[:, 1:2], "2": bna[:, 3:4]}
        if SKIP_PE:  # True or "hybrid"
            # diag(225/inv2): accumulated into conv2 PSUM it contributes
            # sc2 * (225/inv2) * x = 15x, folding the skip-add into the PE
            dssb = cpool.tile([C, C], F32, tag="dssb")
            nc.sync.dma_start(dssb[:], dr["dsc"])
            dsc = cpool.tile([C, C], F32R, tag="dsc")
            nc.vector.tensor_copy(dsc[:], dssb[:])

        # PE warmup: the cost model keeps the PE at a low p-state until it has
        # been continuously busy ~3us. Dependency-free matmuls on zeroed tiles
        # ramp it to full clock while the startup DMAs are in flight.
        wz1 = cpool.tile([1, 1], BF16, tag="wz1")
        nc.gpsimd.memset(wz1[:], 0.0)
        wzr = cpool.tile([1, 512], BF16, tag="wzr")
        nc.gpsimd.memset(wzr[:], 0.0)
        with tc.tile_pool(name="psw", bufs=1, space="PSUM") as ppw:
            psw = ppw.tile([1, 512], F32, tag="psw")
            for _ in range(WARMUP):
                nc.tensor.matmul(psw[:], wz1[:], wzr[:], start=True, stop=True)

        # persistent zero-padded image buffers (borders zeroed once)
        xp_t = [cpool.tile([C, HP, WP], F32R, tag=f"xp{k}", name=f"xp{k}")
                for k in range(NB)]
        a1_t = [cpool.tile([C, HP + 1, WP], FP8, tag=f"a1{k}", name=f"a1{k}")
                for k in range(NB)]
        # zero only the borders (interior is overwritten every image)
        for t in xp_t:
            tf = t[:].bitcast(F32)
            nc.vector.memset(tf[:, 0:1, :], 0.0)
            nc.vector.memset(tf[:, HP - 1:HP, :], 0.0)
            nc.vector.memset(tf[:, :, 0:1], 0.0)
            nc.vector.memset(tf[:, :, WP - 1:WP], 0.0)
        for t in a1_t:
            nc.gpsimd.memset(t[:, 0:1, :], 0.0)
            nc.gpsimd.memset(t[:, HP - 1:HP + 1, :], 0.0)  # rows 33,34 (dummy k-tile)
            nc.gpsimd.memset(t[:, :, 0:1], 0.0)
            nc.gpsimd.memset(t[:, :, WP - 1:WP], 0.0)

        def _front(i, x_skip):
            """load-side of image i: pad-copy, conv1, stage1, conv2 launch."""
            xp = xp_t[i % NB]
            a1 = a1_t[i % NB]

            # pad-copy performs the fp32r pre-rounding for conv1; alternate
            # ACT/DVE so neither becomes the binding engine. Image 0 is
            # latency-critical: split it so conv1-h0 starts after the top rows
            if i == 0:
                nc.scalar.activation(xp[:, 1:19, 1:W + 1], x_skip[:, 0:18, :], AF.Copy)
                nc.vector.tensor_copy(xp[:, 19:H + 1, 1:W + 1], x_skip[:, 18:H, :])
            elif i % 2 == 0:
                nc.scalar.activation(xp[:, 1:H + 1, 1:W + 1], x_skip, AF.Copy)
            else:
                nc.vector.tensor_copy(xp[:, 1:H + 1, 1:W + 1], x_skip)

            # conv1: accumulate 7 taps per 512-pixel half, f32r (1 cyc/row)
            ps1 = [pp1.tile([C, 512], F32, tag="ps", name=f"ps1_{i}_{h}") for h in (0, 1)]
            for h in (0, 1):
                for ti, (ky, kx) in enumerate(TAPS):
                    r0 = 16 * h + ky
                    nc.tensor.matmul(ps1[h][:], w1T[ti],
                                     xp[:, r0:r0 + 16, kx:kx + W],
                                     start=(ti == 0), stop=(ti == len(TAPS) - 1))

            # stage1: a1 = round(clip(s1*inv1 + 15*b1, 0, 15))  (ints 0..15, fp8)
            for h in (0, 1):
                ps1_3 = ps1[h][:].rearrange("c (h w) -> c h w", h=16)
                r = spool.tile([C, 16, W], F32, tag="st_r")
                nc.scalar.activation(r[:], ps1_3, AF.Relu, bias=b_s["1"],
                                     scale=inv1)
                q = spool.tile([C, 16, W], F32, tag="st_q")
                nc.vector.tensor_scalar(q[:], r[:], 15.0, MAGIC, OP.min, OP.add)
                nc.vector.tensor_scalar(a1[:, 1 + 16 * h:17 + 16 * h, 1:W + 1],
                                        q[:], MAGIC, None, OP.subtract)

        def _back(i, x_skip, yout, last=False):
            """store-side of image i: conv2, stage2. Emitted one image behind
            so the in-order PE queue runs conv1(i+1) before conv2(i) and never
            stalls waiting for stage1(i)."""
            a1 = a1_t[i % NB]

            # conv2: exact fp8 integer conv; 3 DoubleRow pair-matmuls + 1 plain
            xp = xp_t[i % NB]
            pe_skip = (SKIP_PE is True or (SKIP_PE == "hybrid" and i % 2 == 0)
                       or (SKIP_PE == "tail" and i >= bl - SKIP_TAIL_N))
            ps2 = [pp2.tile([C, 512], F32, tag="ps", name=f"ps2_{i}_{h}") for h in (0, 1)]
            for h in (0, 1):
                if pe_skip:
                    nc.tensor.matmul(ps2[h][:], dsc[:],
                                     xp[:, 1 + 16 * h:17 + 16 * h, 1:W + 1],
                                     start=True, stop=False)
                if USE_DR:
                    for p, ((ky, kx), _) in enumerate(PAIR2):
                        nc.tensor.matmul(ps2[h][:], wp2[p], _pair_ap(a1, ky, kx, h),
                                         start=(not pe_skip and p == 0),
                                         stop=(p == NPAIR - 1), perf_mode=DR)
                else:
                    flat = [t for pair in PAIR2 for t in pair][:7]
                    for ti, (ky, kx) in enumerate(flat):
                        r0 = 16 * h + ky
                        nc.tensor.matmul(ps2[h][:],
                                         w2sb[:, ti, :].bitcast(FP8),
                                         a1[:, r0:r0 + 16, kx:kx + W],
                                         start=(not pe_skip and ti == 0),
                                         stop=(ti == 6))

            # stage2: out_int = round(clip(sc2*psum + bs2, 0, 15)) as uint8;
            # the fp32->u8 conversion is RNE with low-saturation at 0, so a
            # single min-15 tensor_scalar performs clip+round (HW-verified)
            for h in (0, 1):
                r0 = 16 * h
                ps2_3 = ps2[h][:].rearrange("c (h w) -> c h w", h=16)
                g = spool.tile([C, 16, W], F32, tag="st_g", name=f"g_{i}_{h}")
                nc.scalar.activation(g[:], ps2_3, AF.Identity, bias=b_s["2"],
                                     scale=sc2)
                if pe_skip:
                    src = g
                else:
                    src = spool.tile([C, 16, W], F32, tag="st_h", name=f"hh_{i}_{h}")
                    nc.vector.scalar_tensor_tensor(src[:], x_skip[:, r0:r0 + 16, :],
                                                   15.0, g[:], OP.mult, OP.add)
                nc.vector.tensor_scalar(yout[:, r0:r0 + 16, :], src[:],
                                        15.0, None, OP.min)
                if last:
                    # per-half store from the (idle) ACT hwdge queue
                    nc.scalar.dma_start(dr["y"][i][:, r0:r0 + 16, :],
                                        yout[:, r0:r0 + 16, :])

        def _images():
            # software pipeline with a one-image skew: front(i) then back(i-1)
            pend = {}   # image idx -> (x_skip, yout)
            prev = None

            def flush(k):
                x_skip, yout = pend.pop(k)
                last = k == bl - 1
                _back(k, x_skip, yout, last=last)
                if not last:
                    # per-image store: keeps the tail short
                    nc.sync.dma_start(dr["y"][k], yout)

            for ip in range(bl // 2):
                # one batched in-DMA per image pair (pair 0 preloaded above)
                if ip == 0:
                    xsb2 = xsb2_0
                else:
                    xsb2 = ipool.tile([C, 2, H, W], F32, tag="xsb2")
                    if PAIR_DMA:
                        nc.sync.dma_start(xsb2[:], dr["x"][2 * ip:2 * ip + 2].transpose([1, 0, 2, 3]))
                    else:
                        nc.sync.dma_start(xsb2[:, 0], dr["x"][2 * ip])
                        nc.sync.dma_start(xsb2[:, 1], dr["x"][2 * ip + 1])
                y8 = opool.tile([C, 2, H, W], U8, tag="y8")
                for j in (0, 1):
                    i = 2 * ip + j
                    _front(i, xsb2[:, j])
                    pend[i] = (xsb2[:, j], y8[:, j])
                    if prev is not None:
                        flush(prev)
                    prev = i
            flush(prev)

        if repeat > 1:
            with tc.For_i(0, repeat, 1):
                _images()
        else:
            _images()


def _build(bl=BL, repeat=1):
    nc = bacc.Bacc("TRN2", target_bir_lowering=False, debug=False,
                   enable_asserts=False, num_devices=NCORES)
    dr = {}
    dr["x"] = nc.dram_tensor("x", [bl, C, H, W], F32, kind="ExternalInput").ap()
    dr["w1t"] = nc.dram_tensor("w1t", [C, 7, C], BF16, kind="ExternalInput").ap()
    dr["w2q"] = nc.dram_tensor("w2q", [C, 8, C], U8, kind="ExternalInput").ap()
    dr["bna"] = nc.dram_tensor("bna", [C, 4], F32, kind="ExternalInput").ap()
    dr["dsc"] = nc.dram_tensor("dsc", [C, C], F32, kind="ExternalInput").ap()
    dr["y"] = nc.dram_tensor("y", [bl, C, H, W], U8, kind="ExternalOutput").ap()
    with tile.TileContext(nc) as tc:
        _emit(tc, dr, bl, repeat=repeat)
    nc.compile()
    return nc


_CACHED = None


def _host_quant15(w):
    """DoReFa 4-bit weight quant scaled by 15: odd ints in [-15,15].

    Matches reference bit-for-bit (verified): np.tanh == jax-cpu tanh here,
    np.rint is round-half-to-even like jnp.round.
    """
    t = np.tanh(np.asarray(w, np.float32))
    m = np.float32(np.abs(t).max())
    u = t / (np.float32(2.0) * m) + np.float32(0.5)
    return (2.0 * np.rint(u * np.float32(15.0)) - 15.0).astype(np.float32)


W2ORDER = [t for pair in PAIR2 for t in pair]  # (3,1) dummy -> zeros


def _in_maps(inputs, bl=BL, ncores=NCORES):
    f = lambda v: np.asarray(v, dtype=np.float32)
    x = np.ascontiguousarray(f(inputs["x"]))
    wq1 = _host_quant15(inputs["w1"])   # [O, I, 3, 3]
    wq2 = _host_quant15(inputs["w2"])
    w1t = np.ascontiguousarray(np.stack(
        [wq1[:, :, ky, kx].T for (ky, kx) in TAPS], axis=1).astype(ml_dtypes.bfloat16))
    w2t = np.stack([np.zeros((C, C), np.float32) if ky > 2
                    else wq2[:, :, ky, kx].T for (ky, kx) in W2ORDER], axis=1)
    w2q = np.ascontiguousarray(
        np.asarray(w2t, dtype=ml_dtypes.float8_e4m3fn).view(np.uint8))
    inv1 = f(inputs["gamma1"]) / np.sqrt(f(inputs["var1"]) + np.float32(EPS))
    inv2 = f(inputs["gamma2"]) / np.sqrt(f(inputs["var2"]) + np.float32(EPS))
    bs1 = np.float32(15.0) * f(inputs["beta1"]) - np.float32(15.0) * f(inputs["mean1"]) * inv1
    bs2 = np.float32(15.0) * f(inputs["beta2"]) - np.float32(15.0) * f(inputs["mean2"]) * inv2
    sc2 = inv2 / np.float32(15.0)
    bna = np.ascontiguousarray(np.stack([inv1, bs1, sc2, bs2], axis=1).astype(np.float32))
    dsc = np.ascontiguousarray(np.diag(np.float32(225.0) / inv2).astype(np.float32))
    base = {"w1t": w1t, "w2q": w2q, "bna": bna, "dsc": dsc}
    maps = []
    for c in range(ncores):
        m = dict(base)
        m["x"] = np.ascontiguousarray(x[c * bl:(c + 1) * bl])
        maps.append(m)
    return maps


def _run(inputs, trace=False):
    global _CACHED
    if _CACHED is None:
        _CACHED = _build()
    res = run_bass_kernel_spmd(_CACHED, _in_maps(inputs),
                               core_ids=list(range(NCORES)), trace=trace)
    y8 = np.concatenate([res.results[c]["y"] for c in range(NCORES)], axis=0)
    lut = (np.arange(16, dtype=np.float32) / np.float32(15.0)).astype(np.float32)
    return lut[y8], res


def kernel(**inputs) -> np.ndarray:
    y, _ = _run(inputs, trace=False)
    return y
